# revision 13
# baseline (speedup 1.0000x reference)
"""Trainium2 Bass kernel for FFN-MoE (8 experts, top-2, + shared expert).

Strategy: token-parallel across 8 NeuronCores (4096 tokens each, weights
replicated).  Per core a dense all-expert formulation avoids gather/scatter:
fc1 for every (token, expert) in [feature, token] column-major layout, the
sparse top-2 combine weights folded in as a column scale (broadcast across
partitions via a one-hot matmul), then routed fc2 + shared-expert fc2 + the
combine-weighted b2 term all accumulate into the same PSUM banks.

v4:
  - compute path in bf16 (weights + a dedicated bf16 copy of x): PE fast
    weight load is 2x for 16-bit dtypes (fp32/f32r weights self-load 4B at
    a time), and bf16 error (~0.4% rel) is far inside the 2e-2 gate
  - gate runs on an exact-f32 copy of x (stationary weight-load rounding
    of float32r flips top-2 for ~1e-4 logit margins otherwise)
  - every tensor host-packed so one dma_start with large contiguous
    elements loads it; routing computed per 512-token chunk, software-
    pipelined one chunk ahead of the matmul stages
"""

import numpy as np

import concourse.bacc as bacc
import concourse.mybir as mybir
import concourse.tile as tile
from concourse import bass_utils

# Problem dims (hardcoded per contract).
B, S, H, E, TOPK, DF, SH = 8, 4096, 512, 8, 2, 128, 512
NCORES = 8
T = B * S               # 32768 tokens total
TC = T // NCORES        # 4096 tokens per core
CH = 512                # token chunk (one PSUM bank of fp32)
NCHUNK = TC // CH       # 8
KH = H // 128           # 4 k-tiles over hidden dim
NTT = TC // 128         # 32 token tiles of 128 (for routing)

F32 = mybir.dt.float32
BF16 = mybir.dt.bfloat16
AF = mybir.ActivationFunctionType
ALU = mybir.AluOpType

# Schedule-tuning knobs (PSUM pools + 1 persistent bank must total <= 8).
CFG = {
    "pf1": 3, "py": 2, "pc": 2,
    "xp": 4, "xbp": 4, "s1p": 8, "h1p": 4, "h1sp": 16, "yop": 3,
}


def build_nc(repeat=1):
    nc = bacc.Bacc("TRN2", target_bir_lowering=False, debug=False)

    xh = nc.dram_tensor("xh", [NCHUNK, 128, KH, CH], F32, kind="ExternalInput").ap()
    xbh = nc.dram_tensor("xbh", [NCHUNK, 128, KH, CH], BF16, kind="ExternalInput").ap()
    gwh = nc.dram_tensor("gwh", [128, KH, E], F32, kind="ExternalInput").ap()
    w1h = nc.dram_tensor("w1h", [128, KH, E, DF], BF16, kind="ExternalInput").ap()
    b1h = nc.dram_tensor("b1h", [128, E], F32, kind="ExternalInput").ap()
    w2h = nc.dram_tensor("w2h", [128, E, KH, 128], BF16, kind="ExternalInput").ap()
    b2h = nc.dram_tensor("b2h", [E, H], BF16, kind="ExternalInput").ap()
    sw1h = nc.dram_tensor("sw1h", [128, KH, KH, 128], BF16, kind="ExternalInput").ap()
    sb1h = nc.dram_tensor("sb1h", [128, KH], F32, kind="ExternalInput").ap()
    sw2h = nc.dram_tensor("sw2h", [128, KH, KH, 128], BF16, kind="ExternalInput").ap()
    sb2h = nc.dram_tensor("sb2h", [128, KH], F32, kind="ExternalInput").ap()
    eohh = nc.dram_tensor("eohh", [E, E, DF], BF16, kind="ExternalInput").ap()
    i128 = nc.dram_tensor("i128", [128, 128], F32, kind="ExternalInput").ap()
    yh = nc.dram_tensor("yh", [NCHUNK, 128, KH, CH], F32, kind="ExternalOutput").ap()

    with tile.TileContext(nc) as tc:
        _moe(tc, yh, xh, xbh, gwh, w1h, b1h, w2h, b2h, sw1h, sb1h, sw2h, sb2h,
             eohh, i128, repeat=repeat)
    nc.compile()
    return nc


def _T(tc, frees, shape, dtype, name):
    t, free = tc.tile(shape, dtype, name=name)
    frees.append(free)
    return t


def _moe(tc, yh, xh, xbh, gwh, w1h, b1h, w2h, b2h, sw1h, sb1h, sw2h, sb2h,
         eohh, i128, repeat=1):
    nc = tc.nc
    _frees = []

    # ---------------- persistent SBUF tensors ----------------
    gw_sb = _T(tc, _frees, [128, KH, E], F32, name="gw_sb")
    w1_sb = _T(tc, _frees, [128, KH, E, DF], BF16, name="w1_sb")   # [h_lo,k,e,f]
    b1_sb = _T(tc, _frees, [128, E], F32, name="b1_sb")            # [f, e]
    w2_sb = _T(tc, _frees, [128, E, KH, 128], BF16, name="w2_sb")  # [f,e,m,h']
    b2_sb = _T(tc, _frees, [E, H], BF16, name="b2_sb")             # [e, h']
    sw1_sb = _T(tc, _frees, [128, KH, KH, 128], BF16, name="sw1_sb")
    sb1_sb = _T(tc, _frees, [128, KH], F32, name="sb1_sb")
    sw2_sb = _T(tc, _frees, [128, KH, KH, 128], BF16, name="sw2_sb")
    sb2_sb = _T(tc, _frees, [128, KH], F32, name="sb2_sb")
    eoh_sb = _T(tc, _frees, [E, E, DF], BF16, name="eoh_sb")       # one-hot
    id_sb = _T(tc, _frees, [128, 128], F32, name="id_sb")

    # routing state (whole core shard; only the current chunk's slices hot)
    m8 = _T(tc, _frees, [128, NTT, 8], F32, name="m8")     # sorted top8
    ce = _T(tc, _frees, [128, NTT, E], F32, name="ce")     # combine weights
    cT = _T(tc, _frees, [E, TC], BF16, name="cT")          # c^T [e, t]
    negm1 = _T(tc, _frees, [128, NTT], F32, name="negm1")
    rden = _T(tc, _frees, [128, NTT], F32, name="rden")
    tmpa = _T(tc, _frees, [128, NTT], F32, name="tmpa")

    # persistent PSUM: gate scores (token-major) + ce-transpose staging,
    # packed into ONE bank ([:, :256] scores, [:8, 256:] two transpose slots).
    # Keeps the routing chain off the shared matmul PSUM pools so next-chunk
    # routing overlaps current-chunk compute.
    pmix, _pg_free = tc.tile([128, 512], F32, space="PSUM", name="pmix")
    _frees.append(_pg_free)
    pgate = pmix[:, : NTT * E].rearrange("p (t e) -> p t e", e=E)

    with (
        tc.tile_pool(name="pf1", bufs=CFG["pf1"], space="PSUM") as pf1_pool,
        tc.tile_pool(name="py", bufs=CFG["py"], space="PSUM") as py_pool,
        tc.tile_pool(name="pc", bufs=CFG["pc"], space="PSUM") as pc_pool,
        tc.tile_pool(name="xp", bufs=CFG["xp"]) as xp_pool,
        tc.tile_pool(name="xbp", bufs=CFG["xbp"]) as xb_pool,
        tc.tile_pool(name="s1p", bufs=CFG["s1p"]) as s1_pool,
        tc.tile_pool(name="h1p", bufs=CFG["h1p"]) as h1_pool,
        tc.tile_pool(name="h1sp", bufs=CFG["h1sp"]) as h1s_pool,
        tc.tile_pool(name="yop", bufs=CFG["yop"]) as yo_pool,
        tc.tile_pool(name="mkp", bufs=4) as mk_pool,
    ):
        from contextlib import nullcontext
        loop_cm = tc.For_i(0, repeat, 1) if repeat > 1 else nullcontext()
        with loop_cm:
            xt = {}
            xbt = {}

            def ensure_x(c):
                # f32 copy feeds the gate; bf16 copy feeds the matmuls
                if c not in xt:
                    t = xp_pool.tile([128, KH, CH], F32, tag="xp")
                    nc.sync.dma_start(t, xh[c])
                    xt[c] = t
                    tb = xb_pool.tile([128, KH, CH], BF16, tag="xb")
                    nc.sync.dma_start(tb, xbh[c])
                    xbt[c] = tb
                return xt[c], xbt[c]

            # input DMAs: gate + first x chunk first, then weights by first use
            nc.sync.dma_start(gw_sb, gwh)
            nc.sync.dma_start(id_sb, i128)
            ensure_x(0)
            nc.sync.dma_start(w1_sb, w1h)
            nc.sync.dma_start(b1_sb, b1h)
            ensure_x(1)
            nc.sync.dma_start(sw1_sb, sw1h)
            nc.sync.dma_start(sb1_sb, sb1h)
            nc.sync.dma_start(w2_sb, w2h)
            nc.sync.dma_start(sw2_sb, sw2h)
            nc.sync.dma_start(sb2_sb, sb2h)
            nc.sync.dma_start(b2_sb, b2h)
            nc.sync.dma_start(eoh_sb, eohh)

            def routing(c):
                """gate + top-2 + combine weights + transpose for chunk c."""
                xc, _ = ensure_x(c)
                cs = slice(c * 4, (c + 1) * 4)
                # gate logits: stationary x^T tile in true fp32 (float32r /
                # bf16 weight-load rounding flips top-2 at ~1e-4 margins)
                for s in range(4):
                    tt = c * 4 + s
                    tl = slice(s * 128, (s + 1) * 128)
                    for k in range(KH):
                        nc.tensor.matmul(
                            pgate[:, tt, :],
                            lhsT=xc[:, k, tl],
                            rhs=gw_sb[:, k, :],
                            start=(k == 0),
                            stop=(k == KH - 1),
                        )
                for s in range(4):
                    tt = c * 4 + s
                    nc.vector.max(m8[:, tt, :], pgate[:, tt, :])
                # negm1 = -max1 ; rden = 1 / (1 + exp(max2 - max1))
                nc.vector.tensor_scalar_mul(negm1[:, cs], m8[:, cs, 0], -1.0)
                nc.vector.tensor_tensor(
                    tmpa[:, cs], m8[:, cs, 1], m8[:, cs, 0], op=ALU.subtract
                )
                nc.scalar.activation(tmpa[:, cs], tmpa[:, cs], AF.Exp)
                nc.vector.tensor_scalar_add(tmpa[:, cs], tmpa[:, cs], 1.0)
                nc.vector.reciprocal(rden[:, cs], tmpa[:, cs])
                for s in range(4):
                    tt = c * 4 + s
                    # ce = exp(l - m1) * (l >= m2) * rden
                    nc.scalar.activation(
                        ce[:, tt, :], pgate[:, tt, :], AF.Exp,
                        bias=negm1[:, tt : tt + 1],
                    )
                    # broadcast variants instead of scalar-ptr ops: the DVE
                    # sequencer's per-instruction scalar SBUF read is slow
                    mk = mk_pool.tile([128, E], F32, tag="mk")
                    m2b = m8[:, tt, 1:2].broadcast_to([128, E])
                    rdb = rden[:, tt : tt + 1].broadcast_to([128, E])
                    nc.vector.tensor_tensor(mk, pgate[:, tt, :], m2b, op=ALU.is_ge)
                    nc.vector.tensor_tensor(mk, mk, rdb, op=ALU.mult)
                    nc.vector.tensor_tensor(ce[:, tt, :], ce[:, tt, :], mk, op=ALU.mult)
                    # transpose ce tile -> cT [e, t] (2 rotating slots in pmix)
                    tsg = slice(tt * 128, (tt + 1) * 128)
                    sl = slice(256 + (s % 2) * 128, 256 + (s % 2) * 128 + 128)
                    nc.tensor.matmul(
                        pmix[:E, sl], lhsT=ce[:, tt, :], rhs=id_sb,
                        is_transpose=True, start=True, stop=True,
                    )
                    nc.scalar.copy(cT[:, tsg], pmix[:E, sl])

            routing(0)

            for c in range(NCHUNK):
                _, xb = ensure_x(c)
                if c + 1 < NCHUNK:
                    routing(c + 1)   # overlaps with this chunk's compute
                if c + 2 < NCHUNK:
                    ensure_x(c + 2)
                tok = slice(c * CH, (c + 1) * CH)

                # ======== shared expert fc1 ========
                s1 = []
                for m in range(KH):
                    ps = pf1_pool.tile([128, CH], F32, tag="pf1")
                    for k in range(KH):
                        nc.tensor.matmul(
                            ps, lhsT=sw1_sb[:, k, m, :], rhs=xb[:, k, :],
                            start=(k == 0), stop=(k == KH - 1),
                        )
                    s1m = s1_pool.tile([128, CH], BF16, tag="s1")
                    nc.scalar.activation(s1m, ps, AF.Relu, bias=sb1_sb[:, m : m + 1])
                    s1.append(s1m)

                # ======== routed fc1 + combine-weight fold ========
                h1s = []
                for e in range(E):
                    pf = pf1_pool.tile([128, CH], F32, tag="pf1")
                    for k in range(KH):
                        nc.tensor.matmul(
                            pf, lhsT=w1_sb[:, k, e, :], rhs=xb[:, k, :],
                            start=(k == 0), stop=(k == KH - 1),
                        )
                    h1 = h1_pool.tile([128, CH], F32, tag="h1")
                    nc.scalar.activation(h1, pf, AF.Relu, bias=b1_sb[:, e : e + 1])
                    # broadcast c[:, e] across 128 f-partitions via one-hot matmul
                    pcn = pc_pool.tile([128, CH], F32, tag="pc")
                    nc.tensor.matmul(
                        pcn, lhsT=eoh_sb[:, e, :], rhs=cT[:, tok],
                        start=True, stop=True,
                    )
                    hs_t = h1s_pool.tile([128, CH], BF16, tag="h1s")
                    nc.vector.tensor_tensor(hs_t, h1, pcn, op=ALU.mult)
                    h1s.append(hs_t)

                # ======== fc2: routed + shared + combine-weighted b2 ========
                yo = yo_pool.tile([128, KH, CH], F32, tag="yo")
                for m in range(KH):
                    ms = slice(m * 128, (m + 1) * 128)
                    py = py_pool.tile([128, CH], F32, tag="py")
                    nc.tensor.matmul(
                        py, lhsT=b2_sb[:, ms], rhs=cT[:, tok],
                        start=True, stop=False,
                    )
                    for e in range(E):
                        nc.tensor.matmul(
                            py, lhsT=w2_sb[:, e, m, :], rhs=h1s[e],
                            start=False, stop=False,
                        )
                    for k in range(KH):
                        nc.tensor.matmul(
                            py, lhsT=sw2_sb[:, k, m, :], rhs=s1[k],
                            start=False, stop=(k == KH - 1),
                        )
                    nc.scalar.activation(
                        yo[:, m, :], py, AF.Identity, bias=sb2_sb[:, m : m + 1]
                    )
                nc.sync.dma_start(yh[c], yo)

    for f in reversed(_frees):
        f()


_NC_CACHE = {}


def _get_nc():
    if "nc" not in _NC_CACHE:
        _NC_CACHE["nc"] = build_nc(repeat=1)
    return _NC_CACHE["nc"]


def prepare_in_maps(
    hidden_states, gate_w, w1, b1, w2, b2, sw1, sb1, sw2, sb2
) -> list:
    import ml_dtypes
    bf16 = ml_dtypes.bfloat16

    f32 = lambda a: np.asarray(a, np.float32)
    hidden_states = f32(hidden_states)
    gate_w, w1, b1, w2, b2 = f32(gate_w), f32(w1), f32(b1), f32(w2), f32(b2)
    sw1, sb1, sw2, sb2 = f32(sw1), f32(sb1), f32(sw2), f32(sb2)

    x = hidden_states.reshape(T, H)

    eoh = np.zeros((E, E, DF), bf16)
    for e in range(E):
        eoh[e, e, :] = 1.0

    C = np.ascontiguousarray
    shared = {
        "gwh": C(gate_w.T.reshape(KH, 128, E).transpose(1, 0, 2)),
        "w1h": C(w1.reshape(E, KH, 128, DF).transpose(2, 1, 0, 3)).astype(bf16),
        "b1h": C(b1.T),
        "w2h": C(w2.reshape(E, DF, KH, 128).transpose(1, 0, 2, 3)).astype(bf16),
        "b2h": C(b2).astype(bf16),
        "sw1h": C(sw1.reshape(KH, 128, KH, 128).transpose(1, 0, 2, 3)).astype(bf16),
        "sb1h": C(sb1.reshape(KH, 128).T),
        "sw2h": C(sw2.reshape(KH, 128, KH, 128).transpose(1, 0, 2, 3)).astype(bf16),
        "sb2h": C(sb2.reshape(KH, 128).T),
        "eohh": eoh,
        "i128": np.eye(128, dtype=np.float32),
    }
    in_maps = []
    for c in range(NCORES):
        xc = x[c * TC : (c + 1) * TC]
        xch = C(xc.reshape(NCHUNK, CH, KH, 128).transpose(0, 3, 2, 1))
        in_maps.append({"xh": xch, "xbh": xch.astype(bf16), **shared})
    return in_maps


def kernel(
    hidden_states, gate_w, w1, b1, w2, b2, sw1, sb1, sw2, sb2
) -> np.ndarray:
    in_maps = prepare_in_maps(
        hidden_states, gate_w, w1, b1, w2, b2, sw1, sb1, sw2, sb2
    )

    import os
    # The axon NTFF trace hook is absent in this container; a stray BASS_TRACE
    # env would send run_bass_kernel_spmd down a broken import path.
    os.environ.setdefault("BASS_NEVER_TRACE", "1")
    nc = _get_nc()
    res = bass_utils.run_bass_kernel_spmd(nc, in_maps, core_ids=list(range(NCORES)))
    outs = []
    for r in res.results:
        yhv = np.asarray(r["yh"])  # [NCHUNK, 128, KH, CH]
        outs.append(yhv.transpose(0, 3, 2, 1).reshape(TC, H))
    y = np.concatenate(outs, axis=0)
    return np.ascontiguousarray(y.reshape(B, S, H).astype(np.float32))


if __name__ == "__main__":
    rng = np.random.default_rng(0)
    inputs = {
        "hidden_states": rng.standard_normal((B, S, H)).astype(np.float32),
        "gate_w": (rng.standard_normal((E, H)) * 0.05).astype(np.float32),
        "w1": (rng.standard_normal((E, H, DF)) * 0.05).astype(np.float32),
        "b1": (rng.standard_normal((E, DF)) * 0.01).astype(np.float32),
        "w2": (rng.standard_normal((E, DF, H)) * 0.05).astype(np.float32),
        "b2": (rng.standard_normal((E, H)) * 0.01).astype(np.float32),
        "sw1": (rng.standard_normal((H, SH)) * 0.05).astype(np.float32),
        "sb1": (rng.standard_normal((SH,)) * 0.01).astype(np.float32),
        "sw2": (rng.standard_normal((SH, H)) * 0.05).astype(np.float32),
        "sb2": (rng.standard_normal((H,)) * 0.01).astype(np.float32),
    }
    out = kernel(**inputs)
    print(out.shape, out.dtype, float(np.abs(out).mean()))


# revision 16
# speedup vs baseline: 1.0577x; 1.0577x over previous
"""Trainium2 Bass kernel for FFN-MoE (8 experts, top-2, + shared expert).

Strategy: token-parallel across 8 NeuronCores (4096 tokens each, weights
replicated).  Per core a dense all-expert formulation avoids gather/scatter:
fc1 for every (token, expert) in [feature, token] column-major layout, the
sparse top-2 combine weights folded in as a column scale (broadcast across
partitions via a one-hot matmul), then routed fc2 + shared-expert fc2 + the
combine-weighted b2 term all accumulate into the same PSUM banks.

v4:
  - compute path in bf16 (weights + a dedicated bf16 copy of x): PE fast
    weight load is 2x for 16-bit dtypes (fp32/f32r weights self-load 4B at
    a time), and bf16 error (~0.4% rel) is far inside the 2e-2 gate
  - gate runs on an exact-f32 copy of x (stationary weight-load rounding
    of float32r flips top-2 for ~1e-4 logit margins otherwise)
  - every tensor host-packed so one dma_start with large contiguous
    elements loads it; routing computed per 512-token chunk, software-
    pipelined one chunk ahead of the matmul stages
"""

import numpy as np

import concourse.bacc as bacc
import concourse.mybir as mybir
import concourse.tile as tile
from concourse import bass_utils

# Problem dims (hardcoded per contract).
B, S, H, E, TOPK, DF, SH = 8, 4096, 512, 8, 2, 128, 512
NCORES = 8
T = B * S               # 32768 tokens total
TC = T // NCORES        # 4096 tokens per core
CH = 512                # token chunk (one PSUM bank of fp32)
NCHUNK = TC // CH       # 8
KH = H // 128           # 4 k-tiles over hidden dim
NTT = TC // 128         # 32 token tiles of 128 (for routing)

F32 = mybir.dt.float32
BF16 = mybir.dt.bfloat16
AF = mybir.ActivationFunctionType
ALU = mybir.AluOpType

# Schedule-tuning knobs (PSUM pools + 2 persistent banks must total <= 8).
CFG = {
    "pf1": 2, "py": 2, "pc": 2,
    "xp": 4, "xbp": 4, "s1p": 8, "h1p": 4, "h1sp": 16, "yop": 3,
}


def build_nc(repeat=1):
    nc = bacc.Bacc("TRN2", target_bir_lowering=False, debug=False)

    xh = nc.dram_tensor("xh", [NCHUNK, 128, KH, CH], F32, kind="ExternalInput").ap()
    xbh = nc.dram_tensor("xbh", [NCHUNK, 128, KH, CH], BF16, kind="ExternalInput").ap()
    gwh = nc.dram_tensor("gwh", [128, KH, E], F32, kind="ExternalInput").ap()
    w1h = nc.dram_tensor("w1h", [128, KH, E, DF], BF16, kind="ExternalInput").ap()
    b1h = nc.dram_tensor("b1h", [128, E], F32, kind="ExternalInput").ap()
    w2h = nc.dram_tensor("w2h", [128, E, KH, 128], BF16, kind="ExternalInput").ap()
    b2h = nc.dram_tensor("b2h", [E, H], BF16, kind="ExternalInput").ap()
    sw1h = nc.dram_tensor("sw1h", [128, KH, KH, 128], BF16, kind="ExternalInput").ap()
    sb1h = nc.dram_tensor("sb1h", [128, KH], F32, kind="ExternalInput").ap()
    sw2h = nc.dram_tensor("sw2h", [128, KH, KH, 128], BF16, kind="ExternalInput").ap()
    sb2h = nc.dram_tensor("sb2h", [128, KH], F32, kind="ExternalInput").ap()
    eohh = nc.dram_tensor("eohh", [E, E, DF], BF16, kind="ExternalInput").ap()
    i128 = nc.dram_tensor("i128", [128, 128], F32, kind="ExternalInput").ap()
    yh = nc.dram_tensor("yh", [NCHUNK, 128, KH, CH], F32, kind="ExternalOutput").ap()

    with tile.TileContext(nc) as tc:
        _moe(tc, yh, xh, xbh, gwh, w1h, b1h, w2h, b2h, sw1h, sb1h, sw2h, sb2h,
             eohh, i128, repeat=repeat)
    nc.compile()
    return nc


def _T(tc, frees, shape, dtype, name):
    t, free = tc.tile(shape, dtype, name=name)
    frees.append(free)
    return t


def _moe(tc, yh, xh, xbh, gwh, w1h, b1h, w2h, b2h, sw1h, sb1h, sw2h, sb2h,
         eohh, i128, repeat=1):
    nc = tc.nc
    _frees = []

    # ---------------- persistent SBUF tensors ----------------
    gw_sb = _T(tc, _frees, [128, KH, E], F32, name="gw_sb")
    w1_sb = _T(tc, _frees, [128, KH, E, DF], BF16, name="w1_sb")   # [h_lo,k,e,f]
    b1_sb = _T(tc, _frees, [128, E], F32, name="b1_sb")            # [f, e]
    w2_sb = _T(tc, _frees, [128, E, KH, 128], BF16, name="w2_sb")  # [f,e,m,h']
    b2_sb = _T(tc, _frees, [E, H], BF16, name="b2_sb")             # [e, h']
    sw1_sb = _T(tc, _frees, [128, KH, KH, 128], BF16, name="sw1_sb")
    sb1_sb = _T(tc, _frees, [128, KH], F32, name="sb1_sb")
    sw2_sb = _T(tc, _frees, [128, KH, KH, 128], BF16, name="sw2_sb")
    sb2_sb = _T(tc, _frees, [128, KH], F32, name="sb2_sb")
    eoh_sb = _T(tc, _frees, [E, E, DF], BF16, name="eoh_sb")       # one-hot
    id_sb = _T(tc, _frees, [128, 128], F32, name="id_sb")

    # routing state (whole core shard; only the current chunk's slices hot)
    m8 = _T(tc, _frees, [128, NTT, 8], F32, name="m8")     # sorted top8
    ce = _T(tc, _frees, [128, NTT, E], F32, name="ce")     # combine weights
    cT = _T(tc, _frees, [E, TC], BF16, name="cT")          # c^T [e, t]
    negm1 = _T(tc, _frees, [128, NTT], F32, name="negm1")
    rden = _T(tc, _frees, [128, NTT], F32, name="rden")
    tmpa = _T(tc, _frees, [128, NTT], F32, name="tmpa")

    # persistent PSUM: gate scores (token-major) + ce-transpose staging.
    # Keeps the routing chain off the shared matmul PSUM pools so next-chunk
    # routing overlaps current-chunk compute.
    pgate, _pg_free = tc.tile([128, NTT, E], F32, space="PSUM", name="pgate")
    _frees.append(_pg_free)
    ptr_ps, _pt_free = tc.tile([E, 4, 128], F32, space="PSUM", name="ptr_ps")
    _frees.append(_pt_free)

    with (
        tc.tile_pool(name="pf1", bufs=CFG["pf1"], space="PSUM") as pf1_pool,
        tc.tile_pool(name="py", bufs=CFG["py"], space="PSUM") as py_pool,
        tc.tile_pool(name="pc", bufs=CFG["pc"], space="PSUM") as pc_pool,
        tc.tile_pool(name="xp", bufs=CFG["xp"]) as xp_pool,
        tc.tile_pool(name="xbp", bufs=CFG["xbp"]) as xb_pool,
        tc.tile_pool(name="s1p", bufs=CFG["s1p"]) as s1_pool,
        tc.tile_pool(name="h1p", bufs=CFG["h1p"]) as h1_pool,
        tc.tile_pool(name="h1sp", bufs=CFG["h1sp"]) as h1s_pool,
        tc.tile_pool(name="yop", bufs=CFG["yop"]) as yo_pool,
        tc.tile_pool(name="mkp", bufs=4) as mk_pool,
    ):
        from contextlib import nullcontext
        loop_cm = tc.For_i(0, repeat, 1) if repeat > 1 else nullcontext()
        with loop_cm:
            xt = {}
            xbt = {}

            def ensure_x(c):
                # f32 copy feeds the gate; bf16 copy feeds the matmuls
                if c not in xt:
                    t = xp_pool.tile([128, KH, CH], F32, tag="xp")
                    nc.sync.dma_start(t, xh[c])
                    xt[c] = t
                    tb = xb_pool.tile([128, KH, CH], BF16, tag="xb")
                    nc.sync.dma_start(tb, xbh[c])
                    xbt[c] = tb
                return xt[c], xbt[c]

            # input DMAs: gate + first x chunk first, then weights by first use
            nc.sync.dma_start(gw_sb, gwh)
            nc.sync.dma_start(id_sb, i128)
            ensure_x(0)
            nc.sync.dma_start(w1_sb, w1h)
            nc.sync.dma_start(b1_sb, b1h)
            ensure_x(1)
            nc.sync.dma_start(sw1_sb, sw1h)
            nc.sync.dma_start(sb1_sb, sb1h)
            nc.sync.dma_start(w2_sb, w2h)
            nc.sync.dma_start(sw2_sb, sw2h)
            nc.sync.dma_start(sb2_sb, sb2h)
            nc.sync.dma_start(b2_sb, b2h)
            nc.sync.dma_start(eoh_sb, eohh)

            def routing(c):
                """gate + top-2 + combine weights + transpose for chunk c."""
                xc, _ = ensure_x(c)
                cs = slice(c * 4, (c + 1) * 4)
                # gate logits: stationary x^T tile in true fp32 (float32r /
                # bf16 weight-load rounding flips top-2 at ~1e-4 margins)
                for s in range(4):
                    tt = c * 4 + s
                    tl = slice(s * 128, (s + 1) * 128)
                    for k in range(KH):
                        nc.tensor.matmul(
                            pgate[:, tt, :],
                            lhsT=xc[:, k, tl],
                            rhs=gw_sb[:, k, :],
                            start=(k == 0),
                            stop=(k == KH - 1),
                        )
                for s in range(4):
                    tt = c * 4 + s
                    nc.vector.max(m8[:, tt, :], pgate[:, tt, :])
                # negm1 = -max1 ; rden = 1 / (1 + exp(max2 - max1))
                nc.vector.tensor_scalar_mul(negm1[:, cs], m8[:, cs, 0], -1.0)
                nc.vector.tensor_tensor(
                    tmpa[:, cs], m8[:, cs, 1], m8[:, cs, 0], op=ALU.subtract
                )
                nc.scalar.activation(tmpa[:, cs], tmpa[:, cs], AF.Exp)
                nc.vector.tensor_scalar_add(tmpa[:, cs], tmpa[:, cs], 1.0)
                nc.vector.reciprocal(rden[:, cs], tmpa[:, cs])
                for s in range(4):
                    tt = c * 4 + s
                    # ce = exp(l - m1) * (l >= m2) * rden
                    nc.scalar.activation(
                        ce[:, tt, :], pgate[:, tt, :], AF.Exp,
                        bias=negm1[:, tt : tt + 1],
                    )
                    # broadcast variants instead of scalar-ptr ops: the DVE
                    # sequencer's per-instruction scalar SBUF read is slow
                    mk = mk_pool.tile([128, E], F32, tag="mk")
                    m2b = m8[:, tt, 1:2].broadcast_to([128, E])
                    rdb = rden[:, tt : tt + 1].broadcast_to([128, E])
                    nc.vector.tensor_tensor(mk, pgate[:, tt, :], m2b, op=ALU.is_ge)
                    nc.vector.tensor_tensor(mk, mk, rdb, op=ALU.mult)
                    nc.vector.tensor_tensor(ce[:, tt, :], ce[:, tt, :], mk, op=ALU.mult)
                    # transpose ce tile -> cT [e, t]
                    tsg = slice(tt * 128, (tt + 1) * 128)
                    nc.tensor.matmul(
                        ptr_ps[:, s, :], lhsT=ce[:, tt, :], rhs=id_sb,
                        is_transpose=True, start=True, stop=True,
                    )
                    nc.scalar.copy(cT[:, tsg], ptr_ps[:, s, :])

            routing(0)

            for c in range(NCHUNK):
                _, xb = ensure_x(c)
                if c + 1 < NCHUNK:
                    routing(c + 1)   # overlaps with this chunk's compute
                if c + 2 < NCHUNK:
                    ensure_x(c + 2)
                tok = slice(c * CH, (c + 1) * CH)

                # ======== shared expert fc1 ========
                s1 = []
                for m in range(KH):
                    ps = pf1_pool.tile([128, CH], F32, tag="pf1")
                    for k in range(KH):
                        nc.tensor.matmul(
                            ps, lhsT=sw1_sb[:, k, m, :], rhs=xb[:, k, :],
                            start=(k == 0), stop=(k == KH - 1),
                        )
                    s1m = s1_pool.tile([128, CH], BF16, tag="s1")
                    nc.scalar.activation(s1m, ps, AF.Relu, bias=sb1_sb[:, m : m + 1])
                    s1.append(s1m)

                # ======== routed fc1 + combine-weight fold ========
                h1s = []
                for e in range(E):
                    pf = pf1_pool.tile([128, CH], F32, tag="pf1")
                    for k in range(KH):
                        nc.tensor.matmul(
                            pf, lhsT=w1_sb[:, k, e, :], rhs=xb[:, k, :],
                            start=(k == 0), stop=(k == KH - 1),
                        )
                    h1 = h1_pool.tile([128, CH], F32, tag="h1")
                    nc.scalar.activation(h1, pf, AF.Relu, bias=b1_sb[:, e : e + 1])
                    # broadcast c[:, e] across 128 f-partitions via one-hot matmul
                    pcn = pc_pool.tile([128, CH], F32, tag="pc")
                    nc.tensor.matmul(
                        pcn, lhsT=eoh_sb[:, e, :], rhs=cT[:, tok],
                        start=True, stop=True,
                    )
                    hs_t = h1s_pool.tile([128, CH], BF16, tag="h1s")
                    nc.vector.tensor_tensor(hs_t, h1, pcn, op=ALU.mult)
                    h1s.append(hs_t)

                # ======== fc2: routed + shared + combine-weighted b2 ========
                yo = yo_pool.tile([128, KH, CH], F32, tag="yo")
                for m in range(KH):
                    ms = slice(m * 128, (m + 1) * 128)
                    py = py_pool.tile([128, CH], F32, tag="py")
                    nc.tensor.matmul(
                        py, lhsT=b2_sb[:, ms], rhs=cT[:, tok],
                        start=True, stop=False,
                    )
                    for e in range(E):
                        nc.tensor.matmul(
                            py, lhsT=w2_sb[:, e, m, :], rhs=h1s[e],
                            start=False, stop=False,
                        )
                    for k in range(KH):
                        nc.tensor.matmul(
                            py, lhsT=sw2_sb[:, k, m, :], rhs=s1[k],
                            start=False, stop=(k == KH - 1),
                        )
                    nc.scalar.activation(
                        yo[:, m, :], py, AF.Identity, bias=sb2_sb[:, m : m + 1]
                    )
                nc.sync.dma_start(yh[c], yo)

    for f in reversed(_frees):
        f()


_NC_CACHE = {}


def _get_nc():
    if "nc" not in _NC_CACHE:
        _NC_CACHE["nc"] = build_nc(repeat=1)
    return _NC_CACHE["nc"]


def prepare_in_maps(
    hidden_states, gate_w, w1, b1, w2, b2, sw1, sb1, sw2, sb2
) -> list:
    import ml_dtypes
    bf16 = ml_dtypes.bfloat16

    f32 = lambda a: np.asarray(a, np.float32)
    hidden_states = f32(hidden_states)
    gate_w, w1, b1, w2, b2 = f32(gate_w), f32(w1), f32(b1), f32(w2), f32(b2)
    sw1, sb1, sw2, sb2 = f32(sw1), f32(sb1), f32(sw2), f32(sb2)

    x = hidden_states.reshape(T, H)

    eoh = np.zeros((E, E, DF), bf16)
    for e in range(E):
        eoh[e, e, :] = 1.0

    C = np.ascontiguousarray
    shared = {
        "gwh": C(gate_w.T.reshape(KH, 128, E).transpose(1, 0, 2)),
        "w1h": C(w1.reshape(E, KH, 128, DF).transpose(2, 1, 0, 3)).astype(bf16),
        "b1h": C(b1.T),
        "w2h": C(w2.reshape(E, DF, KH, 128).transpose(1, 0, 2, 3)).astype(bf16),
        "b2h": C(b2).astype(bf16),
        "sw1h": C(sw1.reshape(KH, 128, KH, 128).transpose(1, 0, 2, 3)).astype(bf16),
        "sb1h": C(sb1.reshape(KH, 128).T),
        "sw2h": C(sw2.reshape(KH, 128, KH, 128).transpose(1, 0, 2, 3)).astype(bf16),
        "sb2h": C(sb2.reshape(KH, 128).T),
        "eohh": eoh,
        "i128": np.eye(128, dtype=np.float32),
    }
    in_maps = []
    for c in range(NCORES):
        xc = x[c * TC : (c + 1) * TC]
        xch = C(xc.reshape(NCHUNK, CH, KH, 128).transpose(0, 3, 2, 1))
        in_maps.append({"xh": xch, "xbh": xch.astype(bf16), **shared})
    return in_maps


def kernel(
    hidden_states, gate_w, w1, b1, w2, b2, sw1, sb1, sw2, sb2
) -> np.ndarray:
    in_maps = prepare_in_maps(
        hidden_states, gate_w, w1, b1, w2, b2, sw1, sb1, sw2, sb2
    )

    import os
    # The axon NTFF trace hook is absent in this container; a stray BASS_TRACE
    # env would send run_bass_kernel_spmd down a broken import path.
    os.environ.setdefault("BASS_NEVER_TRACE", "1")
    nc = _get_nc()
    res = bass_utils.run_bass_kernel_spmd(nc, in_maps, core_ids=list(range(NCORES)))
    outs = []
    for r in res.results:
        yhv = np.asarray(r["yh"])  # [NCHUNK, 128, KH, CH]
        outs.append(yhv.transpose(0, 3, 2, 1).reshape(TC, H))
    y = np.concatenate(outs, axis=0)
    return np.ascontiguousarray(y.reshape(B, S, H).astype(np.float32))


if __name__ == "__main__":
    rng = np.random.default_rng(0)
    inputs = {
        "hidden_states": rng.standard_normal((B, S, H)).astype(np.float32),
        "gate_w": (rng.standard_normal((E, H)) * 0.05).astype(np.float32),
        "w1": (rng.standard_normal((E, H, DF)) * 0.05).astype(np.float32),
        "b1": (rng.standard_normal((E, DF)) * 0.01).astype(np.float32),
        "w2": (rng.standard_normal((E, DF, H)) * 0.05).astype(np.float32),
        "b2": (rng.standard_normal((E, H)) * 0.01).astype(np.float32),
        "sw1": (rng.standard_normal((H, SH)) * 0.05).astype(np.float32),
        "sb1": (rng.standard_normal((SH,)) * 0.01).astype(np.float32),
        "sw2": (rng.standard_normal((SH, H)) * 0.05).astype(np.float32),
        "sb2": (rng.standard_normal((H,)) * 0.01).astype(np.float32),
    }
    out = kernel(**inputs)
    print(out.shape, out.dtype, float(np.abs(out).mean()))


# revision 17
# speedup vs baseline: 1.0949x; 1.0352x over previous
"""Trainium2 Bass kernel for FFN-MoE (8 experts, top-2, + shared expert).

Strategy: token-parallel across 8 NeuronCores (4096 tokens each, weights
replicated).  Per core a dense all-expert formulation avoids gather/scatter:
fc1 for every (token, expert) in [feature, token] column-major layout, the
sparse top-2 combine weights folded in as a column scale (broadcast across
partitions via a one-hot matmul), then routed fc2 + shared-expert fc2 + the
combine-weighted b2 term all accumulate into the same PSUM banks.

v4:
  - compute path in bf16 (weights + a dedicated bf16 copy of x): PE fast
    weight load is 2x for 16-bit dtypes (fp32/f32r weights self-load 4B at
    a time), and bf16 error (~0.4% rel) is far inside the 2e-2 gate
  - gate runs on an exact-f32 copy of x (stationary weight-load rounding
    of float32r flips top-2 for ~1e-4 logit margins otherwise)
  - every tensor host-packed so one dma_start with large contiguous
    elements loads it; routing computed per 512-token chunk, software-
    pipelined one chunk ahead of the matmul stages
"""

import numpy as np

import concourse.bacc as bacc
import concourse.mybir as mybir
import concourse.tile as tile
from concourse import bass_utils

# Problem dims (hardcoded per contract).
B, S, H, E, TOPK, DF, SH = 8, 4096, 512, 8, 2, 128, 512
NCORES = 8
T = B * S               # 32768 tokens total
TC = T // NCORES        # 4096 tokens per core
CH = 512                # token chunk (one PSUM bank of fp32)
NCHUNK = TC // CH       # 8
KH = H // 128           # 4 k-tiles over hidden dim
NTT = TC // 128         # 32 token tiles of 128 (for routing)

F32 = mybir.dt.float32
BF16 = mybir.dt.bfloat16
AF = mybir.ActivationFunctionType
ALU = mybir.AluOpType

# Schedule-tuning knobs (PSUM pools + 2 persistent banks must total <= 8).
CFG = {
    "pf1": 2, "py": 2, "pc": 2,
    "xp": 3, "xbp": 3, "s1p": 6, "h1p": 3, "h1sp": 12, "yop": 2,
}


def build_nc(repeat=1):
    nc = bacc.Bacc("TRN2", target_bir_lowering=False, debug=False)

    xh = nc.dram_tensor("xh", [NCHUNK, 128, KH, CH], F32, kind="ExternalInput").ap()
    xbh = nc.dram_tensor("xbh", [NCHUNK, 128, KH, CH], BF16, kind="ExternalInput").ap()
    gwh = nc.dram_tensor("gwh", [128, KH, E], F32, kind="ExternalInput").ap()
    w1h = nc.dram_tensor("w1h", [128, KH, E, DF], BF16, kind="ExternalInput").ap()
    b1h = nc.dram_tensor("b1h", [128, E], F32, kind="ExternalInput").ap()
    w2h = nc.dram_tensor("w2h", [128, E, KH, 128], BF16, kind="ExternalInput").ap()
    b2h = nc.dram_tensor("b2h", [E, H], BF16, kind="ExternalInput").ap()
    sw1h = nc.dram_tensor("sw1h", [128, KH, KH, 128], BF16, kind="ExternalInput").ap()
    sb1h = nc.dram_tensor("sb1h", [128, KH], F32, kind="ExternalInput").ap()
    sw2h = nc.dram_tensor("sw2h", [128, KH, KH, 128], BF16, kind="ExternalInput").ap()
    sb2h = nc.dram_tensor("sb2h", [128, KH], F32, kind="ExternalInput").ap()
    eohh = nc.dram_tensor("eohh", [E, E, DF], BF16, kind="ExternalInput").ap()
    i128 = nc.dram_tensor("i128", [128, 128], F32, kind="ExternalInput").ap()
    yh = nc.dram_tensor("yh", [NCHUNK, 128, KH, CH], F32, kind="ExternalOutput").ap()

    with tile.TileContext(nc) as tc:
        _moe(tc, yh, xh, xbh, gwh, w1h, b1h, w2h, b2h, sw1h, sb1h, sw2h, sb2h,
             eohh, i128, repeat=repeat)
    nc.compile()
    return nc


def _T(tc, frees, shape, dtype, name):
    t, free = tc.tile(shape, dtype, name=name)
    frees.append(free)
    return t


def _moe(tc, yh, xh, xbh, gwh, w1h, b1h, w2h, b2h, sw1h, sb1h, sw2h, sb2h,
         eohh, i128, repeat=1):
    nc = tc.nc
    _frees = []

    # ---------------- persistent SBUF tensors ----------------
    gw_sb = _T(tc, _frees, [128, KH, E], F32, name="gw_sb")
    w1_sb = _T(tc, _frees, [128, KH, E, DF], BF16, name="w1_sb")   # [h_lo,k,e,f]
    b1_sb = _T(tc, _frees, [128, E], F32, name="b1_sb")            # [f, e]
    w2_sb = _T(tc, _frees, [128, E, KH, 128], BF16, name="w2_sb")  # [f,e,m,h']
    b2_sb = _T(tc, _frees, [E, H], BF16, name="b2_sb")             # [e, h']
    sw1_sb = _T(tc, _frees, [128, KH, KH, 128], BF16, name="sw1_sb")
    sb1_sb = _T(tc, _frees, [128, KH], F32, name="sb1_sb")
    sw2_sb = _T(tc, _frees, [128, KH, KH, 128], BF16, name="sw2_sb")
    sb2_sb = _T(tc, _frees, [128, KH], F32, name="sb2_sb")
    eoh_sb = _T(tc, _frees, [E, E, DF], BF16, name="eoh_sb")       # one-hot
    id_sb = _T(tc, _frees, [128, 128], F32, name="id_sb")

    # routing state (whole core shard; only the current chunk's slices hot)
    m8 = _T(tc, _frees, [128, NTT, 8], F32, name="m8")     # sorted top8
    ce = _T(tc, _frees, [128, NTT, E], F32, name="ce")     # combine weights
    cT = _T(tc, _frees, [E, TC], BF16, name="cT")          # c^T [e, t]
    negm1 = _T(tc, _frees, [128, NTT], F32, name="negm1")
    rden = _T(tc, _frees, [128, NTT], F32, name="rden")
    tmpa = _T(tc, _frees, [128, NTT], F32, name="tmpa")

    # persistent PSUM: gate scores (token-major) + ce-transpose staging.
    # Keeps the routing chain off the shared matmul PSUM pools so next-chunk
    # routing overlaps current-chunk compute.
    pgate, _pg_free = tc.tile([128, NTT, E], F32, space="PSUM", name="pgate")
    _frees.append(_pg_free)
    ptr_ps, _pt_free = tc.tile([E, 4, 128], F32, space="PSUM", name="ptr_ps")
    _frees.append(_pt_free)

    with (
        tc.tile_pool(name="pf1", bufs=CFG["pf1"], space="PSUM") as pf1_pool,
        tc.tile_pool(name="py", bufs=CFG["py"], space="PSUM") as py_pool,
        tc.tile_pool(name="pc", bufs=CFG["pc"], space="PSUM") as pc_pool,
        tc.tile_pool(name="xp", bufs=CFG["xp"]) as xp_pool,
        tc.tile_pool(name="xbp", bufs=CFG["xbp"]) as xb_pool,
        tc.tile_pool(name="s1p", bufs=CFG["s1p"]) as s1_pool,
        tc.tile_pool(name="h1p", bufs=CFG["h1p"]) as h1_pool,
        tc.tile_pool(name="h1sp", bufs=CFG["h1sp"]) as h1s_pool,
        tc.tile_pool(name="yop", bufs=CFG["yop"]) as yo_pool,
        tc.tile_pool(name="mkp", bufs=4) as mk_pool,
    ):
        from contextlib import nullcontext
        loop_cm = tc.For_i(0, repeat, 1) if repeat > 1 else nullcontext()
        with loop_cm:
            xt = {}
            xbt = {}

            def ensure_x(c):
                # f32 copy feeds the gate; bf16 copy feeds the matmuls
                if c not in xt:
                    t = xp_pool.tile([128, KH, CH], F32, tag="xp")
                    nc.sync.dma_start(t, xh[c])
                    xt[c] = t
                    tb = xb_pool.tile([128, KH, CH], BF16, tag="xb")
                    nc.sync.dma_start(tb, xbh[c])
                    xbt[c] = tb
                return xt[c], xbt[c]

            # input DMAs: gate + first x chunk first, then weights by first use
            nc.sync.dma_start(gw_sb, gwh)
            nc.sync.dma_start(id_sb, i128)
            ensure_x(0)
            nc.sync.dma_start(w1_sb, w1h)
            nc.sync.dma_start(b1_sb, b1h)
            ensure_x(1)
            nc.sync.dma_start(sw1_sb, sw1h)
            nc.sync.dma_start(sb1_sb, sb1h)
            nc.sync.dma_start(w2_sb, w2h)
            nc.sync.dma_start(sw2_sb, sw2h)
            nc.sync.dma_start(sb2_sb, sb2h)
            nc.sync.dma_start(b2_sb, b2h)
            nc.sync.dma_start(eoh_sb, eohh)

            def routing(c):
                """gate + top-2 + combine weights + transpose for chunk c."""
                xc, _ = ensure_x(c)
                cs = slice(c * 4, (c + 1) * 4)
                # gate logits: stationary x^T tile in true fp32 (float32r /
                # bf16 weight-load rounding flips top-2 at ~1e-4 margins)
                for s in range(4):
                    tt = c * 4 + s
                    tl = slice(s * 128, (s + 1) * 128)
                    for k in range(KH):
                        nc.tensor.matmul(
                            pgate[:, tt, :],
                            lhsT=xc[:, k, tl],
                            rhs=gw_sb[:, k, :],
                            start=(k == 0),
                            stop=(k == KH - 1),
                        )
                for s in range(4):
                    tt = c * 4 + s
                    nc.vector.max(m8[:, tt, :], pgate[:, tt, :])
                # negm1 = -max1 ; rden = 1 / (1 + exp(max2 - max1))
                nc.vector.tensor_scalar_mul(negm1[:, cs], m8[:, cs, 0], -1.0)
                nc.vector.tensor_tensor(
                    tmpa[:, cs], m8[:, cs, 1], m8[:, cs, 0], op=ALU.subtract
                )
                nc.scalar.activation(tmpa[:, cs], tmpa[:, cs], AF.Exp)
                nc.vector.tensor_scalar_add(tmpa[:, cs], tmpa[:, cs], 1.0)
                nc.vector.reciprocal(rden[:, cs], tmpa[:, cs])
                for s in range(4):
                    tt = c * 4 + s
                    # ce = exp(l - m1) * (l >= m2) * rden
                    nc.scalar.activation(
                        ce[:, tt, :], pgate[:, tt, :], AF.Exp,
                        bias=negm1[:, tt : tt + 1],
                    )
                    # broadcast variants instead of scalar-ptr ops: the DVE
                    # sequencer's per-instruction scalar SBUF read is slow
                    mk = mk_pool.tile([128, E], F32, tag="mk")
                    m2b = m8[:, tt, 1:2].broadcast_to([128, E])
                    rdb = rden[:, tt : tt + 1].broadcast_to([128, E])
                    nc.vector.tensor_tensor(mk, pgate[:, tt, :], m2b, op=ALU.is_ge)
                    nc.vector.tensor_tensor(mk, mk, rdb, op=ALU.mult)
                    nc.vector.tensor_tensor(ce[:, tt, :], ce[:, tt, :], mk, op=ALU.mult)
                    # transpose ce tile -> cT [e, t]
                    tsg = slice(tt * 128, (tt + 1) * 128)
                    nc.tensor.matmul(
                        ptr_ps[:, s, :], lhsT=ce[:, tt, :], rhs=id_sb,
                        is_transpose=True, start=True, stop=True,
                    )
                    nc.scalar.copy(cT[:, tsg], ptr_ps[:, s, :])

            routing(0)

            for c in range(NCHUNK):
                _, xb = ensure_x(c)
                if c + 1 < NCHUNK:
                    routing(c + 1)   # overlaps with this chunk's compute
                if c + 2 < NCHUNK:
                    ensure_x(c + 2)
                tok = slice(c * CH, (c + 1) * CH)

                # ======== shared expert fc1 ========
                s1 = []
                for m in range(KH):
                    ps = pf1_pool.tile([128, CH], F32, tag="pf1")
                    for k in range(KH):
                        nc.tensor.matmul(
                            ps, lhsT=sw1_sb[:, k, m, :], rhs=xb[:, k, :],
                            start=(k == 0), stop=(k == KH - 1),
                        )
                    s1m = s1_pool.tile([128, CH], BF16, tag="s1")
                    nc.scalar.activation(s1m, ps, AF.Relu, bias=sb1_sb[:, m : m + 1])
                    s1.append(s1m)

                # ======== routed fc1 + combine-weight fold ========
                h1s = []
                for e in range(E):
                    pf = pf1_pool.tile([128, CH], F32, tag="pf1")
                    for k in range(KH):
                        nc.tensor.matmul(
                            pf, lhsT=w1_sb[:, k, e, :], rhs=xb[:, k, :],
                            start=(k == 0), stop=(k == KH - 1),
                        )
                    h1 = h1_pool.tile([128, CH], F32, tag="h1")
                    nc.scalar.activation(h1, pf, AF.Relu, bias=b1_sb[:, e : e + 1])
                    # broadcast c[:, e] across 128 f-partitions via one-hot matmul
                    pcn = pc_pool.tile([128, CH], F32, tag="pc")
                    nc.tensor.matmul(
                        pcn, lhsT=eoh_sb[:, e, :], rhs=cT[:, tok],
                        start=True, stop=True,
                    )
                    hs_t = h1s_pool.tile([128, CH], BF16, tag="h1s")
                    nc.vector.tensor_tensor(hs_t, h1, pcn, op=ALU.mult)
                    h1s.append(hs_t)

                # ======== fc2: routed + shared + combine-weighted b2 ========
                yo = yo_pool.tile([128, KH, CH], F32, tag="yo")
                for m in range(KH):
                    ms = slice(m * 128, (m + 1) * 128)
                    py = py_pool.tile([128, CH], F32, tag="py")
                    nc.tensor.matmul(
                        py, lhsT=b2_sb[:, ms], rhs=cT[:, tok],
                        start=True, stop=False,
                    )
                    for e in range(E):
                        nc.tensor.matmul(
                            py, lhsT=w2_sb[:, e, m, :], rhs=h1s[e],
                            start=False, stop=False,
                        )
                    for k in range(KH):
                        nc.tensor.matmul(
                            py, lhsT=sw2_sb[:, k, m, :], rhs=s1[k],
                            start=False, stop=(k == KH - 1),
                        )
                    nc.scalar.activation(
                        yo[:, m, :], py, AF.Identity, bias=sb2_sb[:, m : m + 1]
                    )
                nc.sync.dma_start(yh[c], yo)

    for f in reversed(_frees):
        f()


_NC_CACHE = {}


def _get_nc():
    if "nc" not in _NC_CACHE:
        _NC_CACHE["nc"] = build_nc(repeat=1)
    return _NC_CACHE["nc"]


def prepare_in_maps(
    hidden_states, gate_w, w1, b1, w2, b2, sw1, sb1, sw2, sb2
) -> list:
    import ml_dtypes
    bf16 = ml_dtypes.bfloat16

    f32 = lambda a: np.asarray(a, np.float32)
    hidden_states = f32(hidden_states)
    gate_w, w1, b1, w2, b2 = f32(gate_w), f32(w1), f32(b1), f32(w2), f32(b2)
    sw1, sb1, sw2, sb2 = f32(sw1), f32(sb1), f32(sw2), f32(sb2)

    x = hidden_states.reshape(T, H)

    eoh = np.zeros((E, E, DF), bf16)
    for e in range(E):
        eoh[e, e, :] = 1.0

    C = np.ascontiguousarray
    shared = {
        "gwh": C(gate_w.T.reshape(KH, 128, E).transpose(1, 0, 2)),
        "w1h": C(w1.reshape(E, KH, 128, DF).transpose(2, 1, 0, 3)).astype(bf16),
        "b1h": C(b1.T),
        "w2h": C(w2.reshape(E, DF, KH, 128).transpose(1, 0, 2, 3)).astype(bf16),
        "b2h": C(b2).astype(bf16),
        "sw1h": C(sw1.reshape(KH, 128, KH, 128).transpose(1, 0, 2, 3)).astype(bf16),
        "sb1h": C(sb1.reshape(KH, 128).T),
        "sw2h": C(sw2.reshape(KH, 128, KH, 128).transpose(1, 0, 2, 3)).astype(bf16),
        "sb2h": C(sb2.reshape(KH, 128).T),
        "eohh": eoh,
        "i128": np.eye(128, dtype=np.float32),
    }
    in_maps = []
    for c in range(NCORES):
        xc = x[c * TC : (c + 1) * TC]
        xch = C(xc.reshape(NCHUNK, CH, KH, 128).transpose(0, 3, 2, 1))
        in_maps.append({"xh": xch, "xbh": xch.astype(bf16), **shared})
    return in_maps


def kernel(
    hidden_states, gate_w, w1, b1, w2, b2, sw1, sb1, sw2, sb2
) -> np.ndarray:
    in_maps = prepare_in_maps(
        hidden_states, gate_w, w1, b1, w2, b2, sw1, sb1, sw2, sb2
    )

    import os
    # The axon NTFF trace hook is absent in this container; a stray BASS_TRACE
    # env would send run_bass_kernel_spmd down a broken import path.
    os.environ.setdefault("BASS_NEVER_TRACE", "1")
    nc = _get_nc()
    res = bass_utils.run_bass_kernel_spmd(nc, in_maps, core_ids=list(range(NCORES)))
    outs = []
    for r in res.results:
        yhv = np.asarray(r["yh"])  # [NCHUNK, 128, KH, CH]
        outs.append(yhv.transpose(0, 3, 2, 1).reshape(TC, H))
    y = np.concatenate(outs, axis=0)
    return np.ascontiguousarray(y.reshape(B, S, H).astype(np.float32))


if __name__ == "__main__":
    rng = np.random.default_rng(0)
    inputs = {
        "hidden_states": rng.standard_normal((B, S, H)).astype(np.float32),
        "gate_w": (rng.standard_normal((E, H)) * 0.05).astype(np.float32),
        "w1": (rng.standard_normal((E, H, DF)) * 0.05).astype(np.float32),
        "b1": (rng.standard_normal((E, DF)) * 0.01).astype(np.float32),
        "w2": (rng.standard_normal((E, DF, H)) * 0.05).astype(np.float32),
        "b2": (rng.standard_normal((E, H)) * 0.01).astype(np.float32),
        "sw1": (rng.standard_normal((H, SH)) * 0.05).astype(np.float32),
        "sb1": (rng.standard_normal((SH,)) * 0.01).astype(np.float32),
        "sw2": (rng.standard_normal((SH, H)) * 0.05).astype(np.float32),
        "sb2": (rng.standard_normal((H,)) * 0.01).astype(np.float32),
    }
    out = kernel(**inputs)
    print(out.shape, out.dtype, float(np.abs(out).mean()))
